# revision 13
# baseline (speedup 1.0000x reference)
"""Trainium2 Bass kernel for nn_Attention_44830868635854.

Fused: 1x1-conv QKV -> depthwise 3x3 on q -> 8-head attention (softmax) ->
ReLU -> 1x1 proj -> GroupNorm(8).

Sharding: 8 cores = (batch b in 0..3) x (spatial half s in 0..1). Each core
computes output rows [24s, 24s+24) of the 48x48 image for its batch (n_slice
= 1152 pixels) across all 8 heads, using the full image for k/v (attention
is global). GroupNorm statistics are combined across the core pair with a
tiny AllReduce.

Layout key: heads are processed in 2 groups of 4. Within a group, head jj
occupies partitions 32*jj..32*jj+15 (its 16 channels); attention logits are
computed transposed (partition = key position m, free = query position n) so
softmax needs no reductions: the exp'd P matrix feeds a matmul against
[v^T | ones] which yields both the unnormalized output O and the softmax
denominator S in one pass. Normalization happens once per output tile.
"""

import numpy as np

import concourse.bass as bass
import concourse.mybir as mybir
import concourse.tile as tile
from concourse.tile import add_dep_helper
from concourse.bass_utils import run_bass_kernel_spmd

F32 = mybir.dt.float32
F32R = mybir.dt.float32r
BF16 = mybir.dt.bfloat16
AF = mybir.ActivationFunctionType
ALU = mybir.AluOpType

B, DIM, H, W = 4, 128, 48, 48
HEADS, HEAD_DIM = 8, 16
N = H * W            # 2304
ROWS_HALF = 24
NSL = ROWS_HALF * W  # 1152 per core
NT = 384             # n-tile (3 per core)
MT = 128             # m-tile (18 per core)
EPS = 1e-5
GN_DIV = 1.0 / (16.0 * N)


def _split_multi_waits(nc):
    """walrus here allows one sync-wait slot per lowered instruction; move
    extra waits onto standalone EventSemaphore instructions."""
    for func in nc.m.functions:
        for block in func.blocks:
            new_insts = []
            for inst in block.instructions:
                si = inst.sync_info
                waits = list(si.on_wait) if si is not None and si.on_wait else []
                if len(waits) > 1 and not isinstance(inst, mybir.InstEventSemaphore):
                    for k, w in enumerate(waits[:-1]):
                        new_insts.append(
                            mybir.InstEventSemaphore(
                                name=f"{inst.name}_wsplit{k}",
                                engine=inst.engine,
                                ins=[],
                                outs=[],
                                sync_info=mybir.SyncInfo(on_wait=[w], on_update=[]),
                            )
                        )
                    si.on_wait = waits[-1:]
                new_insts.append(inst)
            block.instructions[:] = new_insts


def _build():
    nc = bass.Bass()
    dt = nc.dram_tensor

    xb_d = dt("xb", [DIM, N], F32, kind="ExternalInput")
    xq_d = dt("xq", [DIM, 26 * W], F32, kind="ExternalInput")
    wk_d = dt("wk", [2, DIM, 128], F32, kind="ExternalInput")
    wq_d = dt("wq", [2, DIM, 128], F32, kind="ExternalInput")
    wv_d = dt("wv", [2, DIM, 128], F32, kind="ExternalInput")
    bvr_d = dt("bvr", [2, 128, 128], F32, kind="ExternalInput")
    bq_d = dt("bq", [2, 128, 1], F32, kind="ExternalInput")
    bdw_d = dt("bdw", [2, 128, 1], F32, kind="ExternalInput")
    wdw_d = dt("wdw", [2, 128, 9], F32, kind="ExternalInput")
    wpj_d = dt("wpj", [2, DIM, 128], F32, kind="ExternalInput")
    gab_d = dt("gab", [DIM, 2], F32, kind="ExternalInput")  # gn gamma | beta
    gsel_d = dt("gsel", [DIM, 8], F32, kind="ExternalInput")

    out_d = dt("out_half", [DIM, NSL], F32, kind="ExternalOutput")
    dbg_att_d = dt("dbg_att", [2, DIM, NSL], F32, kind="ExternalOutput")
    dbg_o2_d = dt("dbg_o2", [DIM, NSL], F32, kind="ExternalOutput")
    dbg_st_d = dt("dbg_st", [DIM, 4], F32, kind="ExternalOutput")
    dbg_q_d = dt("dbg_q", [2, DIM, NSL], F32, kind="ExternalOutput")
    dbg_k_d = dt("dbg_k", [2, DIM, N], F32, kind="ExternalOutput")
    dbg_acc_d = dt("dbg_acc", [2, DIM, NT], F32, kind="ExternalOutput")

    cc_in = dt("cc_in", [8, 2], F32)
    cc_out = dt("cc_out", [8, 2], F32)
    r_dram = dt("r_dram", [6, 4, NT], F32)
    scratch_d = dt("scratch", [128, 1], F32)

    with tile.TileContext(nc) as tc:
        with (
            tc.tile_pool(name="persist", bufs=1) as pp,
            tc.tile_pool(name="work", bufs=2) as wk2,
            tc.tile_pool(name="ppool", bufs=3) as wp3,
            tc.tile_pool(name="lp", bufs=2, space="PSUM") as lpp,
        ):
            # ---- ACT exp table preload (single-wait discipline for hot loop)
            dummy = pp.tile([128, 1], F32, tag="dummy")
            nc.vector.memset(dummy, 0.0)
            nc.scalar.activation(out=dummy, in_=dummy, func=AF.Exp)
            nc.gpsimd.dma_start(out=scratch_d[:, :], in_=dummy)

            # ---- load inputs
            xb = pp.tile([DIM, N], F32, tag="xb")
            nc.gpsimd.dma_start(out=xb, in_=xb_d[:, :])
            xbr = pp.tile([DIM, N], F32R, tag="xbr")
            nc.vector.tensor_copy(out=xbr, in_=xb)

            xq = pp.tile([DIM, 26 * W], F32, tag="xq")
            nc.gpsimd.dma_start(out=xq, in_=xq_d[:, :])
            xqr = pp.tile([DIM, 26 * W], F32R, tag="xqr")
            nc.vector.tensor_copy(out=xqr, in_=xq)

            wkr, wqr, wvr, wpjr = [], [], [], []
            bvr, bqv, bdwv, wdwv = [], [], [], []
            for g in range(2):
                t = pp.tile([DIM, 128], F32, tag=f"wk{g}")
                nc.gpsimd.dma_start(out=t, in_=wk_d[g, :, :])
                tr = pp.tile([DIM, 128], F32R, tag=f"wkr{g}")
                nc.vector.tensor_copy(out=tr, in_=t)
                wkr.append(tr)
                t = pp.tile([DIM, 128], F32, tag=f"wq{g}")
                nc.gpsimd.dma_start(out=t, in_=wq_d[g, :, :])
                tr = pp.tile([DIM, 128], F32R, tag=f"wqr{g}")
                nc.vector.tensor_copy(out=tr, in_=t)
                wqr.append(tr)
                t = pp.tile([DIM, 128], F32, tag=f"wv{g}")
                nc.gpsimd.dma_start(out=t, in_=wv_d[g, :, :])
                tr = pp.tile([DIM, 128], F32R, tag=f"wvr{g}")
                nc.vector.tensor_copy(out=tr, in_=t)
                wvr.append(tr)
                t = pp.tile([DIM, 128], F32, tag=f"wpj{g}")
                nc.gpsimd.dma_start(out=t, in_=wpj_d[g, :, :])
                tr = pp.tile([DIM, 128], F32R, tag=f"wpjr{g}")
                nc.vector.tensor_copy(out=tr, in_=t)
                wpjr.append(tr)
                t = pp.tile([128, 128], F32, tag=f"bvr{g}")
                nc.gpsimd.dma_start(out=t, in_=bvr_d[g, :, :])
                bvr.append(t)
                t = pp.tile([128, 1], F32, tag=f"bq{g}")
                nc.gpsimd.dma_start(out=t, in_=bq_d[g, :, :])
                bqv.append(t)
                t = pp.tile([128, 1], F32, tag=f"bdw{g}")
                nc.gpsimd.dma_start(out=t, in_=bdw_d[g, :, :])
                bdwv.append(t)
                t = pp.tile([128, 9], F32, tag=f"wdw{g}")
                nc.gpsimd.dma_start(out=t, in_=wdw_d[g, :, :])
                wdwv.append(t)
            gab = pp.tile([DIM, 2], F32, tag="gab")
            nc.gpsimd.dma_start(out=gab, in_=gab_d[:, :])
            gsel = pp.tile([DIM, 8], F32, tag="gsel")
            nc.gpsimd.dma_start(out=gsel, in_=gsel_d[:, :])
            ones17f = pp.tile([DIM, 17], F32, tag="ones17f")
            nc.vector.memset(ones17f, 1.0)

            # ---- k projection: k_g [128, N] fp32r (head jj at rows 32jj..+15)
            kg = []
            for g in range(2):
                kt = pp.tile([DIM, N], F32R, tag=f"kg{g}")
                for j0 in range(0, N, 512):
                    n = min(512, N - j0)
                    ps = lpp.tile([128, 4, 512], F32, tag="lp")
                    nc.tensor.matmul(
                        out=ps[:, 0, 0:n], lhsT=wkr[g], rhs=xbr[:, j0 : j0 + n],
                        start=True, stop=True,
                    )
                    nc.vector.tensor_copy(out=kt[:, j0 : j0 + n], in_=ps[:, 0, 0:n])
                kg.append(kt)

            # ---- v^T tiles: vt[g][i] [128(m), 128] bf16, cols 32jj+d = v dim,
            #      col 32jj+16 = 1 (from bias tile), rest 0
            vt = [[None] * (N // MT) for _ in range(2)]
            for i in range(N // MT):
                for g in range(2):
                    ps = lpp.tile([128, 4, 512], F32, tag="lp")
                    nc.tensor.matmul(
                        out=ps[:, 0, 0:128], lhsT=xbr[:, i * MT : (i + 1) * MT],
                        rhs=wvr[g], start=True, stop=True,
                    )
                    t = pp.tile([128, 128], BF16, tag=f"vt{g}_{i}")
                    nc.vector.tensor_add(out=t, in0=ps[:, 0, 0:128], in1=bvr[g])
                    vt[g][i] = t

            # ---- q: project 26 rows, add bias into padded buf, dw conv 3x3
            qg = []
            for g in range(2):
                qp = pp.tile([128, 26 * 50], F32, tag=f"qp{g}")
                nc.vector.memset(qp, 0.0)
                qpv = qp.rearrange("p (r c) -> p r c", c=50)
                for r0 in range(0, 26, 9):  # row blocks 9,9,8 -> N=432,432,384
                    nr = min(9, 26 - r0)
                    ps = lpp.tile([128, 4, 512], F32, tag="lp")
                    nc.tensor.matmul(
                        out=ps[:, 0, 0 : nr * W],
                        lhsT=wqr[g],
                        rhs=xqr[:, r0 * W : (r0 + nr) * W],
                        start=True, stop=True,
                    )
                    nc.vector.tensor_scalar_add(
                        out=qpv[:, r0 : r0 + nr, 1 : 1 + W],
                        in0=ps[:, 0, 0 : nr * W].rearrange("p (r c) -> p r c", c=W),
                        scalar1=bqv[g],
                    )
                qt = pp.tile([128, NSL], F32R, tag=f"qg{g}")
                qs = pp.tile([128, NSL], F32, tag=f"qs{g}")
                first = True
                for ty in range(3):
                    for tx in range(3):
                        tap = 3 * ty + tx
                        src = qpv[:, ty : ty + 24, tx : tx + W]
                        if first:
                            nc.vector.tensor_scalar_mul(
                                out=qs, in0=src, scalar1=wdwv[g][:, tap : tap + 1]
                            )
                            first = False
                        elif tap < 8:
                            nc.vector.scalar_tensor_tensor(
                                out=qs, in0=src, scalar=wdwv[g][:, tap : tap + 1],
                                in1=qs, op0=ALU.mult, op1=ALU.add,
                            )
                        else:
                            nc.vector.scalar_tensor_tensor(
                                out=qs, in0=src, scalar=wdwv[g][:, tap : tap + 1],
                                in1=qs, op0=ALU.mult, op1=ALU.add,
                            )
                nc.vector.tensor_scalar_add(out=qt, in0=qs, scalar1=bdwv[g])
                qg.append(qt)

            # ---- attention main loop
            att = []
            for g in range(2):
                a = pp.tile([DIM, NSL], F32R, tag=f"att{g}")
                nc.vector.memset(a.bitcast(F32), 0.0)
                att.append(a)

            for g in range(2):
                for j in range(NSL // NT):
                    js = slice(j * NT, (j + 1) * NT)
                    acc = wk2.tile([128, NT], F32, tag="acc")
                    for i in range(N // MT):
                        lp = lpp.tile([128, 4, 512], F32, tag="lp")
                        for jj in range(4):
                            nc.tensor.matmul(
                                out=lp[:, jj, 0:NT],
                                lhsT=kg[g][32 * jj : 32 * jj + 16,
                                           i * MT : (i + 1) * MT],
                                rhs=qg[g][32 * jj : 32 * jj + 16, js],
                                start=True, stop=True,
                                tile_position=(32 * jj, 0),
                            )
                        pt = wp3.tile([128, 4, NT], BF16, tag="pt")
                        nc.scalar.activation(
                            out=pt, in_=lp[:, :, 0:NT], func=AF.Exp, scale=0.25
                        )
                        for jj in range(4):
                            nc.tensor.matmul(
                                out=lp[32 * jj : 32 * jj + 32, 0, 0:NT],
                                lhsT=vt[g][i][:, 32 * jj : 32 * jj + 32],
                                rhs=pt[:, jj, :],
                                start=True, stop=True,
                                tile_position=(0, 32 * jj),
                            )
                        if i == 0:
                            nc.vector.tensor_copy(out=acc, in_=lp[:, 0, 0:NT])
                        else:
                            nc.vector.tensor_add(
                                out=acc, in0=acc, in1=lp[:, 0, 0:NT]
                            )
                    # finalize (g, j): broadcast S (row 32jj) across the
                    # head's rows via a K=1 f32 matmul, reciprocate, multiply
                    rbp = lpp.tile([128, 4, 512], F32, tag="lp")
                    for jj in range(4):
                        nc.tensor.matmul(
                            out=rbp[32 * jj : 32 * jj + 17, 1, 0:NT],
                            lhsT=ones17f[32 * jj : 32 * jj + 1, :],
                            rhs=acc[32 * jj : 32 * jj + 1, :],
                            start=True, stop=True,
                            tile_position=(32 * jj, 32 * jj),
                        )
                    rrec = wk2.tile([128, NT], F32, tag="rrec")
                    nc.vector.reciprocal(out=rrec, in_=rbp[:, 1, 0:NT])
                    for jj in range(4):
                        nc.vector.tensor_mul(
                            out=att[g][32 * jj : 32 * jj + 17, js],
                            in0=acc[32 * jj : 32 * jj + 17, :],
                            in1=rrec[32 * jj : 32 * jj + 17, :],
                        )
                    nc.vector.tensor_scalar_max(
                        out=att[g][:, js], in0=att[g][:, js], scalar1=0.0
                    )

            for g in range(2):
                nc.gpsimd.dma_start(out=dbg_att_d[g, :, :], in_=att[g].bitcast(F32))

            # ---- proj + GroupNorm
            o2 = pp.tile([DIM, NSL], F32, tag="o2")
            for j in range(NSL // NT):
                js = slice(j * NT, (j + 1) * NT)
                pj = lpp.tile([128, 4, 512], F32, tag="lp")
                for g in range(2):
                    nc.tensor.matmul(
                        out=pj[:, 0, 0:NT], lhsT=wpjr[g], rhs=att[g][:, js],
                        start=(g == 0), stop=(g == 1),
                    )
                nc.vector.tensor_copy(out=o2[:, js], in_=pj[:, 0, 0:NT])

            s12 = pp.tile([DIM, 2], F32, tag="s12")
            nc.vector.tensor_reduce(
                out=s12[:, 0:1], in_=o2, op=ALU.add, axis=mybir.AxisListType.X
            )
            sq = pp.tile([DIM, NSL], F32, tag="sq")
            nc.vector.tensor_mul(out=sq, in0=o2, in1=o2)
            nc.vector.tensor_reduce(
                out=s12[:, 1:2], in_=sq, op=ALU.add, axis=mybir.AxisListType.X
            )
            s12r = pp.tile([DIM, 2], F32R, tag="s12r")
            nc.vector.tensor_copy(out=s12r, in_=s12)
            gselr = pp.tile([DIM, 8], F32R, tag="gselr")
            nc.vector.tensor_copy(out=gselr, in_=gsel)
            gp = lpp.tile([128, 4, 512], F32, tag="lp")
            nc.tensor.matmul(
                out=gp[0:8, 0, 0:2], lhsT=gselr, rhs=s12r, start=True, stop=True
            )
            gst = pp.tile([8, 2], F32, tag="gst")
            nc.vector.tensor_copy(out=gst, in_=gp[0:8, 0, 0:2])
            ccw = nc.gpsimd.dma_start(out=cc_in[:, :], in_=gst)
            cci = nc.gpsimd.collective_compute(
                "AllReduce", ALU.add,
                ins=[cc_in[:, :]], outs=[cc_out[:, :]],
                replica_groups=[[0, 1], [2, 3], [4, 5], [6, 7]],
            )
            add_dep_helper(cci.ins, ccw.ins, reason="cc_in RAW")
            gch = pp.tile([DIM, 2], F32, tag="gch")
            ccr = nc.gpsimd.dma_start(
                out=gch,
                in_=bass.AP(
                    tensor=cc_out[:, :].tensor, offset=0,
                    ap=[[2, 8], [0, 16], [1, 2]],
                ),
            )
            add_dep_helper(ccr.ins, cci.ins, reason="cc_out RAW")
            # mu, var -> rstd = exp(-0.5*ln(var+eps)); A = rstd*gamma;
            # Bc = beta - mu*A; out = o2*A + Bc
            mu = pp.tile([DIM, 1], F32, tag="mu")
            nc.vector.tensor_scalar_mul(out=mu, in0=gch[:, 0:1], scalar1=GN_DIV)
            ex2 = pp.tile([DIM, 1], F32, tag="ex2")
            nc.vector.tensor_scalar_mul(out=ex2, in0=gch[:, 1:2], scalar1=GN_DIV)
            mu2 = pp.tile([DIM, 1], F32, tag="mu2")
            nc.vector.tensor_mul(out=mu2, in0=mu, in1=mu)
            var = pp.tile([DIM, 1], F32, tag="var")
            nc.vector.tensor_sub(out=var, in0=ex2, in1=mu2)
            epst = pp.tile([DIM, 1], F32, tag="epst")
            nc.vector.memset(epst, EPS)
            lnv = pp.tile([DIM, 1], F32, tag="lnv")
            nc.scalar.activation(out=lnv, in_=var, func=AF.Ln, bias=epst)
            rstd = pp.tile([DIM, 1], F32, tag="rstd")
            nc.scalar.activation(out=rstd, in_=lnv, func=AF.Exp, scale=-0.5)
            A = pp.tile([DIM, 1], F32, tag="A")
            nc.vector.tensor_mul(out=A, in0=rstd, in1=gab[:, 0:1])
            muA = pp.tile([DIM, 1], F32, tag="muA")
            nc.vector.tensor_mul(out=muA, in0=mu, in1=A)
            Bc = pp.tile([DIM, 1], F32, tag="Bc")
            nc.vector.tensor_sub(out=Bc, in0=gab[:, 1:2], in1=muA)
            nc.gpsimd.dma_start(out=dbg_o2_d[:, :], in_=o2)
            dst = pp.tile([DIM, 4], F32, tag="dst")
            nc.vector.tensor_copy(out=dst[:, 0:1], in_=mu)
            nc.vector.tensor_copy(out=dst[:, 1:2], in_=var)
            nc.vector.tensor_copy(out=dst[:, 2:3], in_=rstd)
            nc.vector.tensor_copy(out=dst[:, 3:4], in_=gch[:, 0:1])
            nc.gpsimd.dma_start(out=dbg_st_d[:, :], in_=dst)
            of = pp.tile([DIM, NSL], F32, tag="of")
            nc.vector.tensor_scalar(
                out=of, in0=o2, scalar1=A, scalar2=Bc,
                op0=ALU.mult, op1=ALU.add,
            )
            nc.gpsimd.dma_start(out=out_d[:, :], in_=of)

    _split_multi_waits(nc)
    return nc


_CACHE = {}


def _prep(w_qkv, b_qkv, w_dw, b_dw, w_proj, gn_w, gn_b):
    """Host-side weight layout prep (group g, slot jj in 0..3, dim d)."""
    ch = lambda g, jj, d: (4 * g + jj) * 16 + d
    wk = np.zeros((2, DIM, 128), np.float32)
    wq = np.zeros((2, DIM, 128), np.float32)
    wv = np.zeros((2, DIM, 128), np.float32)
    bvr = np.zeros((2, 128, 128), np.float32)
    bq = np.zeros((2, 128, 1), np.float32)
    bdw = np.zeros((2, 128, 1), np.float32)
    wdw = np.zeros((2, 128, 9), np.float32)
    wpj = np.zeros((2, DIM, 128), np.float32)
    for g in range(2):
        for jj in range(4):
            for d in range(16):
                c = ch(g, jj, d)
                p = 32 * jj + d
                wq[g, :, p] = w_qkv[c, :]
                wk[g, :, p] = w_qkv[128 + c, :]
                wv[g, :, p + 1] = w_qkv[256 + c, :]
                bvr[g, :, p + 1] = b_qkv[256 + c]
                bq[g, p, 0] = b_qkv[c]
                bdw[g, p, 0] = b_dw[c]
                wdw[g, p, :] = w_dw[c, 0].reshape(9)
                wpj[g, p + 1, :] = w_proj[:, c]
            bvr[g, :, 32 * jj] = 1.0
    gab = np.stack([gn_w, gn_b], axis=1).astype(np.float32)
    gsel = np.zeros((DIM, 8), np.float32)
    for c in range(DIM):
        gsel[c, c // 16] = 1.0
    # pad pixel x-vector: projects exactly to -b_q so bias-add yields 0
    vpad = -np.linalg.solve(w_qkv[0:128, :].astype(np.float64),
                            b_qkv[0:128].astype(np.float64)).astype(np.float32)
    return dict(wk=wk, wq=wq, wv=wv, bvr=bvr, bq=bq, bdw=bdw, wdw=wdw,
                wpj=wpj, gab=gab, gsel=gsel), vpad


def kernel(x, w_qkv, b_qkv, w_dw, b_dw, w_proj, gn_w, gn_b):
    x = np.asarray(x, np.float32)
    w_qkv = np.asarray(w_qkv, np.float32)
    b_qkv = np.asarray(b_qkv, np.float32)
    w_dw = np.asarray(w_dw, np.float32)
    b_dw = np.asarray(b_dw, np.float32)
    w_proj = np.asarray(w_proj, np.float32)
    gn_w = np.asarray(gn_w, np.float32)
    gn_b = np.asarray(gn_b, np.float32)

    weights, vpad = _prep(w_qkv, b_qkv, w_dw, b_dw, w_proj, gn_w, gn_b)

    if "nc" not in _CACHE:
        _CACHE["nc"] = _build()
    nc = _CACHE["nc"]

    in_maps = []
    for c in range(8):
        b, s = c // 2, c % 2
        xb = x[b].reshape(DIM, N)
        # q source: image rows 24s-1 .. 24s+24, boundary row = vpad columns
        xq = np.empty((DIM, 26, W), np.float32)
        xv = x[b]  # [DIM, H, W]
        if s == 0:
            xq[:, 0, :] = vpad[:, None]
            xq[:, 1:26, :] = xv[:, 0:25, :]
        else:
            xq[:, 0:25, :] = xv[:, 23:48, :]
            xq[:, 25, :] = vpad[:, None]
        m = {"xb": np.ascontiguousarray(xb),
             "xq": xq.reshape(DIM, 26 * W)}
        m.update(weights)
        in_maps.append(m)

    res = run_bass_kernel_spmd(nc, in_maps, core_ids=list(range(8)))

    out = np.empty((B, DIM, H, W), np.float32)
    for c in range(8):
        b, s = c // 2, c % 2
        out[b, :, 24 * s : 24 * s + 24, :] = res.results[c]["out_half"].reshape(
            DIM, ROWS_HALF, W
        )
    return out
